# revision 11
# baseline (speedup 1.0000x reference)
"""Causal ReLU-attention block (qkv proj + per-head attention) on 8 trn2 cores.

Sharding: pure data-parallel over batch (B=8 -> 1 batch element per core).
Per-core: x_b [T,C] -> qkv -> scoresT = k q^T (row-tiled head pairs) ->
relu/causal-mask -> yT = v.T @ scoresT (col-tiled head pairs) -> DRAM yT [C,T].
Host side: transpose/cast shards in, transpose gather out.

Schedule: the qk projection for the first head-pair group runs as a short
prelude; every remaining projection matmul (later qk o-tiles + all v tiles)
is interleaved into the attention super-steps as background PE work so the
eviction engines (ACT/DVE) hide under matmul time instead of pacing the
kernel. The causal mask is fused into the diagonal-block eviction as
max(s,0) * mask{scale,0} (one DVE scalar_tensor_tensor), replacing the
per-block gpsimd affine_select. Input DMAs: x on the scalar queue (free
early), W on sync, host-permuted into PE consumption order so every DMA
has >=1.5KB contiguous runs; y DMAs ride sync after W is in.
"""

import sys
from collections import deque
from contextlib import ExitStack

sys.path.insert(0, "/opt/trn_rl_repo")

import ml_dtypes
import numpy as np

import concourse.bass as bass
import concourse.tile as tile
from concourse import bacc, bass_utils, mybir

P = 128
QW = 512  # t_q chunk width (PSUM bank = 512 fp32)

BF16 = mybir.dt.bfloat16
F32 = mybir.dt.float32
AF = mybir.ActivationFunctionType
ALU = mybir.AluOpType

# qk o-tiles (12 of them over [q|k] = 2C) in emission order: the q and k
# tiles of head pair hp are needed together, group (hp, hp+1) at a time.
OT_ORDER = [0, 6, 1, 7, 2, 8, 3, 9, 4, 10, 5, 11]
NSLOT = 18  # 12 qk slots + 6 v slots (128 cols each)
# W slot DMA order: group0 qk, v-g0, group1 qk, v-g1, group2 qk, v-g2
W_DMA_ORDER = [0, 1, 2, 3, 12, 13, 4, 5, 6, 7, 14, 15, 8, 9, 10, 11, 16, 17]


def build_module(T=1024, C=768, H=12, n_cores=8):
    """Build + compile the per-core Bass module (same program on all cores)."""
    hd = C // H
    assert hd == 64 and H % 2 == 0 and C % P == 0 and T % QW == 0
    CT = C // P            # contraction tiles over C
    TT = T // P            # t tiles
    NQC = T // QW          # q chunks
    NHP = H // 2           # head pairs
    scale = 1.0 / float(np.sqrt(hd))

    nc = bacc.Bacc("TRN2", target_bir_lowering=False, debug=False,
                   num_devices=n_cores)

    xT = nc.dram_tensor("xT", [C, T], BF16, kind="ExternalInput").ap()
    # W pre-permuted on host: [p, slot, ct, 128] (see _prep_in_maps)
    wP = nc.dram_tensor("wP", [P, NSLOT, CT, P], BF16, kind="ExternalInput").ap()
    bqk = nc.dram_tensor("bqk", [P, 2 * CT], F32, kind="ExternalInput").ap()
    bv = nc.dram_tensor("bv", [P, C], F32, kind="ExternalInput").ap()
    yT = nc.dram_tensor("yT", [C, T], BF16, kind="ExternalOutput").ap()

    xT3 = xT.rearrange("(ct p) t -> p ct t", p=P)

    with tile.TileContext(nc) as tc, ExitStack() as ctx:
        const = ctx.enter_context(tc.tile_pool(name="const", bufs=1))
        psum = ctx.enter_context(tc.tile_pool(name="psum", bufs=3, space="PSUM"))
        ypsum = ctx.enter_context(tc.tile_pool(name="ypsum", bufs=2, space="PSUM"))
        scb = ctx.enter_context(tc.tile_pool(name="scb", bufs=10))
        ysb = ctx.enter_context(tc.tile_pool(name="ysb", bufs=2))

        wt_sb = const.tile([P, NSLOT, CT, P], BF16)
        xt_sb = const.tile([P, CT, T], BF16)
        bqk_sb = const.tile([P, 2 * CT], F32)
        bv_sb = const.tile([P, C], F32)

        # ---- input DMAs --------------------------------------------------
        # Per-queue DMA bandwidth is only ~100-160GB/s, so the early feed is
        # split across all three DMA-capable queues: x full rows (2KB
        # contiguous runs) alternate scalar (even ct) / gpsimd (odd ct) in
        # consumption order; W slots ride sync, then sync later takes y.
        for qc in range(NQC):
            for ct in range(CT):
                eng = nc.scalar if ct % 2 == 0 else nc.gpsimd
                eng.dma_start(xt_sb[:, ct, qc * QW:(qc + 1) * QW],
                              xT3[:, ct, qc * QW:(qc + 1) * QW])
        nc.scalar.dma_start(bqk_sb[:], bqk[:])
        nc.scalar.dma_start(bv_sb[:], bv[:])
        for s in W_DMA_ORDER:
            nc.sync.dma_start(wt_sb[:, s], wP[:, s])

        # causal mask for diagonal 128x128 strips of scoresT [t_k, t_q]:
        # mask[p, j] = scale where j >= p else 0 (relu(scale*s)*m ==
        # max(s,0)*(scale*m) since scale > 0)
        mask_sc = const.tile([P, P], BF16)
        nc.gpsimd.memset(mask_sc[:], scale)
        nc.gpsimd.affine_select(
            mask_sc[:], mask_sc[:], pattern=[[1, P]],
            compare_op=ALU.is_ge, fill=0.0, base=0, channel_multiplier=-1)

        qkT = const.tile([P, 2 * CT, T], BF16)   # o-tiles: q = 0..CT-1, k = CT..
        vsb = const.tile([P, TT, C], BF16)       # v in natural [t, o] layout

        evict = [0]

        def relu_evict(dst, src):
            # relu(scale * s): PSUM -> SBUF bf16; ACT is 1.25x faster than
            # DVE so give it 3 of every 5 (DVE also owns the fused diagonal
            # evictions and v bias adds)
            if evict[0] % 5 < 3:
                nc.scalar.activation(dst, src, AF.Relu, scale=scale)
            else:
                nc.vector.tensor_scalar(dst, src, scale, 0.0, ALU.mult, ALU.max)
            evict[0] += 1

        # ---- background (projection) chains ------------------------------
        # Each chain is atomic (alloc ... evict in one pop): a PSUM buf may
        # only be held across instructions emitted before its eviction, else
        # the round-robin pool can deadlock the tensor FIFO.
        def qk_chain(slot):
            """One qk o-tile: 2 qc x CT-deep accumulation chains into one
            PSUM tile, evicted by ACT with the fused per-partition bias."""
            ot = OT_ORDER[slot]

            def fn():
                ps = psum.tile([P, NQC, QW], F32, tag="blk", name="qk_ps")
                for qc in range(NQC):
                    for ct in range(CT):
                        nc.tensor.matmul(
                            ps[:, qc],
                            wt_sb[:, slot, ct, :],
                            xt_sb[:, ct, qc * QW:(qc + 1) * QW],
                            start=(ct == 0), stop=(ct == CT - 1),
                        )
                if evict[0] % 2 == 0:
                    nc.scalar.activation(
                        qkT[:, ot], ps.rearrange("p a b -> p (a b)"),
                        AF.Identity, bias=bqk_sb[:, ot:ot + 1])
                else:
                    nc.vector.tensor_scalar(
                        qkT[:, ot], ps.rearrange("p a b -> p (a b)"),
                        bqk_sb[:, ot:ot + 1], None, ALU.add)
                evict[0] += 1

            return (NQC * CT * QW, fn)

        def v_part(g, tt):
            """v columns [g*256, (g+1)*256) for one t-tile: 6-deep chain +
            bias add (DVE; PSUM-reading tensor_tensor is DVE-only)."""
            def fn():
                ps = psum.tile([P, 2, P], F32, tag="blk", name="v_ps")
                for ct in range(CT):
                    nc.tensor.matmul(
                        ps[:],
                        xt_sb[:, ct, tt * P:(tt + 1) * P],
                        wt_sb[:, 12 + 2 * g:14 + 2 * g, ct, :],
                        start=(ct == 0), stop=(ct == CT - 1),
                    )
                nc.vector.tensor_tensor(
                    vsb[:, tt, g * 2 * P:(g + 1) * 2 * P],
                    ps.rearrange("p a b -> p (a b)"),
                    bv_sb[:, g * 2 * P:(g + 1) * 2 * P], ALU.add)
            return (CT * 2 * P, fn)

        # ---- attention ----------------------------------------------------
        def attention_closures(hp):
            """Parallel (scores, att@v) emission closures per block step for
            one head pair; the interleaver runs att@v a full super-step
            behind its scores so the FIFO PE queue always has ready work."""
            items = []
            for qc in range(NQC):
                kb_hi = min((qc * QW + QW - 1) // P, TT - 1)
                for kb in range(kb_hi + 1):
                    items.append((qc, kb, kb_hi))
            state = {"s": {}, "y": {}}
            sc_fns, av_fns = [], []

            def sc(i, qc, kb, kb_hi):
                delta = max(kb * P - qc * QW, 0)   # first valid t_q col
                sp = psum.tile([P, 2, QW], F32, tag="blk", name="s_ps")
                for h, ppos in ((0, (0, 0)), (1, (64, 0))):
                    nc.tensor.matmul(
                        sp[:, h, delta:QW],
                        qkT[h * 64:(h + 1) * 64, CT + hp,
                            kb * P:(kb + 1) * P],
                        qkT[h * 64:(h + 1) * 64, hp,
                            qc * QW + delta:(qc + 1) * QW],
                        start=True, stop=True, tile_position=ppos,
                    )
                s = scb.tile([P, 2, QW], BF16, tag="s")
                if kb * P >= qc * QW:
                    # diagonal block: fused relu+scale+causal-mask on the
                    # first P cols (row p only masks j' < p < P)
                    nc.vector.scalar_tensor_tensor(
                        s[:, :, delta:delta + P],
                        sp[:, :, delta:delta + P],
                        0.0,
                        mask_sc[:, None, :].to_broadcast((P, 2, P)),
                        ALU.max, ALU.mult)
                    if delta + P < QW:
                        relu_evict(s[:, :, delta + P:QW],
                                   sp[:, :, delta + P:QW])
                else:
                    relu_evict(s[:, :, delta:QW], sp[:, :, delta:QW])
                state["s"][i] = s

            def av(i, qc, kb, kb_hi):
                if kb == 0:
                    state["y"][qc] = ypsum.tile([P, QW], F32, tag="y",
                                                name="yp")
                yp = state["y"][qc]
                delta = max(kb * P - qc * QW, 0)
                s = state["s"].pop(i)
                # the two heads accumulate into disjoint partition ranges of
                # one bank (different per-partition SRAMs, so concurrent
                # drains are safe); each runs its own start/stop group (the
                # sim's group checker can't see base partition -> skip)
                nc.tensor.matmul(
                    yp[0:64, delta:QW], vsb[:, kb, hp * P:hp * P + 64],
                    s[:, 0, delta:QW],
                    start=(kb == 0), stop=(kb == kb_hi),
                    tile_position=(0, 0), skip_group_check=True,
                )
                nc.tensor.matmul(
                    yp[64:128, delta:QW],
                    vsb[:, kb, hp * P + 64:hp * P + 128],
                    s[:, 1, delta:QW],
                    start=(kb == 0), stop=(kb == kb_hi),
                    tile_position=(0, 64), skip_group_check=True,
                )
                if kb == kb_hi:
                    yp = state["y"].pop(qc)
                    yt = ysb.tile([P, QW], BF16, tag="yt")
                    # one full-partition eviction: engine cost scales with
                    # free-dim cols only, so splitting by partition halves
                    # would double the engine time
                    if evict[0] % 2 == 0:
                        nc.scalar.activation(yt[:], yp[:], AF.Copy)
                    else:
                        nc.vector.tensor_copy(yt[:], yp[:])
                    evict[0] += 1
                    nc.sync.dma_start(
                        yT[hp * P:(hp + 1) * P, qc * QW:(qc + 1) * QW],
                        yt[:])

            for i, (qc, kb, kb_hi) in enumerate(items):
                sc_fns.append(
                    lambda i=i, qc=qc, kb=kb, kb_hi=kb_hi: sc(i, qc, kb, kb_hi))
                av_fns.append(
                    lambda i=i, qc=qc, kb=kb, kb_hi=kb_hi: av(i, qc, kb, kb_hi))
            return sc_fns, av_fns

        # ---- schedule -----------------------------------------------------
        # Prelude: qk o-tiles for group 0 (heads 0-3), paced to DMA arrival.
        # x rows land ~[ct0, ct2, ct1, ct4, ct3, ct5] (scalar/gpsimd split),
        # w slots land s0..s3 in order, so slots 0-2 interleave mms in that
        # arrival order (3 PSUM bufs) and slot 3 runs once data is resident.
        pre_ps, pre_seen = {}, {}

        def pre_mm(slot, ct, qc):
            if slot not in pre_ps:
                pre_ps[slot] = psum.tile([P, NQC, QW], F32, tag="blk",
                                         name="qk_ps")
            ps = pre_ps[slot]
            seen = pre_seen.setdefault(slot, set())
            nc.tensor.matmul(
                ps[:, qc],
                wt_sb[:, slot, ct, :],
                xt_sb[:, ct, qc * QW:(qc + 1) * QW],
                start=(ct == 0), stop=(ct == CT - 1),
            )
            seen.add((ct, qc))
            if len(seen) == CT * NQC:
                nc.scalar.activation(
                    qkT[:, OT_ORDER[slot]], ps.rearrange("p a b -> p (a b)"),
                    AF.Identity, bias=bqk_sb[:, OT_ORDER[slot]:OT_ORDER[slot] + 1])
                del pre_ps[slot]

        for qc in range(NQC):
            for ct in range(CT):
                for slot in range(3):
                    pre_mm(slot, ct, qc)
        for qc in range(NQC):
            for ct in range(CT):
                pre_mm(3, ct, qc)

        # Per-window background: v chains for the *current* group pop
        # aggressively (2/step) since av step kb needs v tile kb; the next
        # group's qk tiles follow a cycle budget.
        groups = [(0, 1), (2, 3), (4, 5)]
        LAG = 3
        for g, grp in enumerate(groups):
            streams = [attention_closures(hp) for hp in grp]
            front = deque(v_part(g, tt) for tt in range(TT))
            rest = deque()
            if g < 2:
                for slot in range(4 + 4 * g, 8 + 4 * g):
                    rest.append(qk_chain(slot))
            n = len(streams[0][0])
            nsteps = n + LAG
            budget = sum(c for c, _ in rest)
            spent = 0
            for i in range(nsteps):
                for _ in range(2):
                    if front:
                        front.popleft()[1]()
                while rest and spent < (i + 1) * budget // nsteps:
                    cyc, fn = rest.popleft()
                    fn()
                    spent += cyc
                if i < n:
                    for sc_fns, _ in streams:
                        sc_fns[i]()
                if i >= LAG:
                    for _, av_fns in streams:
                        av_fns[i - LAG]()
            while front:
                front.popleft()[1]()
            for cyc, fn in rest:
                fn()

    nc.compile()
    return nc


_CACHE = {}


def _get_module():
    if "nc" not in _CACHE:
        _CACHE["nc"] = build_module()
    return _CACHE["nc"]


def _prep_in_maps(x, W_attn, b_attn, T=1024, C=768, n_cores=8):
    bf = ml_dtypes.bfloat16
    CT = C // P
    OT = 2 * C // P
    WT = W_attn.astype(np.float32).T                                   # [C, 3C]
    # permute columns into PE consumption order: 12 qk o-tiles in OT_ORDER,
    # then the 6 v column tiles; lay out as [p, slot, ct, 128] so each slot
    # is 1.5KB-contiguous per partition for the DMA.
    cols = np.concatenate(
        [np.arange(ot * P, (ot + 1) * P) for ot in OT_ORDER]
        + [np.arange(2 * C, 3 * C)])
    wPm = WT[:, cols].reshape(CT, P, NSLOT, P).transpose(1, 2, 0, 3)
    wPm = np.ascontiguousarray(wPm).astype(bf)              # [P, 18, CT, 128]
    bqk = np.ascontiguousarray(
        b_attn[:2 * C].astype(np.float32).reshape(OT, P).T)            # [P, OT]
    bv = np.ascontiguousarray(
        np.tile(b_attn[2 * C:].astype(np.float32)[None, :], (P, 1)))   # [P, C]
    in_maps = []
    for c in range(n_cores):
        xT_b = np.ascontiguousarray(x[c].astype(np.float32).T).astype(bf)
        in_maps.append({"xT": xT_b, "wP": wPm, "bqk": bqk, "bv": bv})
    return in_maps


def run(x, W_attn, b_attn, trace=False):
    nc = _get_module()
    in_maps = _prep_in_maps(x, W_attn, b_attn)
    res = bass_utils.run_bass_kernel_spmd(
        nc, in_maps, core_ids=list(range(8)), trace=trace)
    y = np.stack([np.asarray(res.results[c]["yT"]).T for c in range(8)])
    return np.ascontiguousarray(y.astype(np.float32)), res


def kernel(x, W_attn, b_attn):
    y, _ = run(x, W_attn, b_attn, trace=False)
    return y


# revision 12
# speedup vs baseline: 1.0167x; 1.0167x over previous
"""Causal ReLU-attention block (qkv proj + per-head attention) on 8 trn2 cores.

Sharding: pure data-parallel over batch (B=8 -> 1 batch element per core).
Per-core: x_b [T,C] -> qkv -> scoresT = k q^T (row-tiled head pairs) ->
relu/causal-mask -> yT = v.T @ scoresT (col-tiled head pairs) -> DRAM yT [C,T].
Host side: transpose/cast shards in, transpose gather out.

Schedule: the qk projection for the first head-pair group runs as a short
prelude; every remaining projection matmul (later qk o-tiles + all v tiles)
is interleaved into the attention super-steps as background PE work so the
eviction engines (ACT/DVE) hide under matmul time instead of pacing the
kernel. The causal mask is fused into the diagonal-block eviction as
max(s,0) * mask{scale,0} (one DVE scalar_tensor_tensor), replacing the
per-block gpsimd affine_select. Input DMAs: x on the scalar queue (free
early), W on sync, host-permuted into PE consumption order so every DMA
has >=1.5KB contiguous runs; y DMAs ride sync after W is in.
"""

import sys
from collections import deque
from contextlib import ExitStack

sys.path.insert(0, "/opt/trn_rl_repo")

import ml_dtypes
import numpy as np

import concourse.bass as bass
import concourse.tile as tile
from concourse import bacc, bass_utils, mybir

P = 128
QW = 512  # t_q chunk width (PSUM bank = 512 fp32)

BF16 = mybir.dt.bfloat16
F32 = mybir.dt.float32
AF = mybir.ActivationFunctionType
ALU = mybir.AluOpType

# qk o-tiles (12 of them over [q|k] = 2C) in emission order: the q and k
# tiles of head pair hp are needed together, group (hp, hp+1) at a time.
OT_ORDER = [0, 6, 1, 7, 2, 8, 3, 9, 4, 10, 5, 11]
NSLOT = 18  # 12 qk slots + 6 v slots (128 cols each)
# W slot DMA order: group0 qk, v-g0, group1 qk, v-g1, group2 qk, v-g2
W_DMA_ORDER = [0, 1, 2, 3, 12, 13, 4, 5, 6, 7, 14, 15, 8, 9, 10, 11, 16, 17]


def build_module(T=1024, C=768, H=12, n_cores=8):
    """Build + compile the per-core Bass module (same program on all cores)."""
    hd = C // H
    assert hd == 64 and H % 2 == 0 and C % P == 0 and T % QW == 0
    CT = C // P            # contraction tiles over C
    TT = T // P            # t tiles
    NQC = T // QW          # q chunks
    NHP = H // 2           # head pairs
    scale = 1.0 / float(np.sqrt(hd))

    nc = bacc.Bacc("TRN2", target_bir_lowering=False, debug=False,
                   num_devices=n_cores)

    xT = nc.dram_tensor("xT", [C, T], BF16, kind="ExternalInput").ap()
    # W pre-permuted on host: [p, slot, ct, 128] (see _prep_in_maps)
    wP = nc.dram_tensor("wP", [P, NSLOT, CT, P], BF16, kind="ExternalInput").ap()
    bqk = nc.dram_tensor("bqk", [P, 2 * CT], F32, kind="ExternalInput").ap()
    bv = nc.dram_tensor("bv", [P, C], F32, kind="ExternalInput").ap()
    yT = nc.dram_tensor("yT", [C, T], BF16, kind="ExternalOutput").ap()

    xT3 = xT.rearrange("(ct p) t -> p ct t", p=P)

    with tile.TileContext(nc) as tc, ExitStack() as ctx:
        const = ctx.enter_context(tc.tile_pool(name="const", bufs=1))
        psum = ctx.enter_context(tc.tile_pool(name="psum", bufs=3, space="PSUM"))
        ypsum = ctx.enter_context(tc.tile_pool(name="ypsum", bufs=2, space="PSUM"))
        scb = ctx.enter_context(tc.tile_pool(name="scb", bufs=10))
        ysb = ctx.enter_context(tc.tile_pool(name="ysb", bufs=2))

        wt_sb = const.tile([P, NSLOT, CT, P], BF16)
        xt_sb = const.tile([P, CT, T], BF16)
        bqk_sb = const.tile([P, 2 * CT], F32)
        bv_sb = const.tile([P, C], F32)

        # ---- input DMAs --------------------------------------------------
        # Per-queue DMA bandwidth is only ~100-160GB/s, so the early feed is
        # split across all three DMA-capable queues: x full rows (2KB
        # contiguous runs) alternate scalar (even ct) / gpsimd (odd ct) in
        # consumption order; W slots ride sync, then sync later takes y.
        for qc in range(NQC):
            for ct in range(CT):
                eng = nc.scalar if ct % 2 == 0 else nc.gpsimd
                eng.dma_start(xt_sb[:, ct, qc * QW:(qc + 1) * QW],
                              xT3[:, ct, qc * QW:(qc + 1) * QW])
        nc.scalar.dma_start(bqk_sb[:], bqk[:])
        nc.scalar.dma_start(bv_sb[:], bv[:])
        for s in W_DMA_ORDER:
            nc.sync.dma_start(wt_sb[:, s], wP[:, s])

        # causal mask for diagonal 128x128 strips of scoresT [t_k, t_q]:
        # mask[p, j] = scale where j >= p else 0 (relu(scale*s)*m ==
        # max(s,0)*(scale*m) since scale > 0)
        mask_sc = const.tile([P, P], BF16)
        nc.gpsimd.memset(mask_sc[:], scale)
        nc.gpsimd.affine_select(
            mask_sc[:], mask_sc[:], pattern=[[1, P]],
            compare_op=ALU.is_ge, fill=0.0, base=0, channel_multiplier=-1)

        qkT = const.tile([P, 2 * CT, T], BF16)   # o-tiles: q = 0..CT-1, k = CT..
        vsb = const.tile([P, TT, C], BF16)       # v in natural [t, o] layout

        evict = [0]

        def relu_evict(dst, src):
            # relu(scale * s): PSUM -> SBUF bf16; ACT is 1.25x faster than
            # DVE so give it 3 of every 5 (DVE also owns the fused diagonal
            # evictions and v bias adds)
            if evict[0] % 5 < 3:
                nc.scalar.activation(dst, src, AF.Relu, scale=scale)
            else:
                nc.vector.tensor_scalar(dst, src, scale, 0.0, ALU.mult, ALU.max)
            evict[0] += 1

        # ---- background (projection) chains ------------------------------
        # Each chain is atomic (alloc ... evict in one pop): a PSUM buf may
        # only be held across instructions emitted before its eviction, else
        # the round-robin pool can deadlock the tensor FIFO.
        def qk_chain(slot):
            """One qk o-tile: 2 qc x CT-deep accumulation chains into one
            PSUM tile, evicted by ACT with the fused per-partition bias."""
            ot = OT_ORDER[slot]

            def fn():
                ps = psum.tile([P, NQC, QW], F32, tag="blk", name="qk_ps")
                for qc in range(NQC):
                    for ct in range(CT):
                        nc.tensor.matmul(
                            ps[:, qc],
                            wt_sb[:, slot, ct, :],
                            xt_sb[:, ct, qc * QW:(qc + 1) * QW],
                            start=(ct == 0), stop=(ct == CT - 1),
                        )
                nc.scalar.activation(
                    qkT[:, ot], ps.rearrange("p a b -> p (a b)"),
                    AF.Identity, bias=bqk_sb[:, ot:ot + 1])

            return (NQC * CT * QW, fn)

        def v_part(g, tt):
            """v columns [g*256, (g+1)*256) for one t-tile: 6-deep chain +
            bias add (DVE; PSUM-reading tensor_tensor is DVE-only)."""
            def fn():
                ps = psum.tile([P, 2, P], F32, tag="blk", name="v_ps")
                for ct in range(CT):
                    nc.tensor.matmul(
                        ps[:],
                        xt_sb[:, ct, tt * P:(tt + 1) * P],
                        wt_sb[:, 12 + 2 * g:14 + 2 * g, ct, :],
                        start=(ct == 0), stop=(ct == CT - 1),
                    )
                nc.vector.tensor_tensor(
                    vsb[:, tt, g * 2 * P:(g + 1) * 2 * P],
                    ps.rearrange("p a b -> p (a b)"),
                    bv_sb[:, g * 2 * P:(g + 1) * 2 * P], ALU.add)
            return (CT * 2 * P, fn)

        # ---- attention ----------------------------------------------------
        def attention_closures(hp):
            """Parallel (scores, att@v) emission closures per block step for
            one head pair; the interleaver runs att@v a full super-step
            behind its scores so the FIFO PE queue always has ready work."""
            items = []
            for qc in range(NQC):
                kb_hi = min((qc * QW + QW - 1) // P, TT - 1)
                for kb in range(kb_hi + 1):
                    items.append((qc, kb, kb_hi))
            state = {"s": {}, "y": {}}
            sc_fns, av_fns = [], []

            def sc(i, qc, kb, kb_hi):
                delta = max(kb * P - qc * QW, 0)   # first valid t_q col
                sp = psum.tile([P, 2, QW], F32, tag="blk", name="s_ps")
                for h, ppos in ((0, (0, 0)), (1, (64, 0))):
                    nc.tensor.matmul(
                        sp[:, h, delta:QW],
                        qkT[h * 64:(h + 1) * 64, CT + hp,
                            kb * P:(kb + 1) * P],
                        qkT[h * 64:(h + 1) * 64, hp,
                            qc * QW + delta:(qc + 1) * QW],
                        start=True, stop=True, tile_position=ppos,
                    )
                s = scb.tile([P, 2, QW], BF16, tag="s")
                if kb * P >= qc * QW:
                    # diagonal block: fused relu+scale+causal-mask on the
                    # first P cols (row p only masks j' < p < P)
                    nc.vector.scalar_tensor_tensor(
                        s[:, :, delta:delta + P],
                        sp[:, :, delta:delta + P],
                        0.0,
                        mask_sc[:, None, :].to_broadcast((P, 2, P)),
                        ALU.max, ALU.mult)
                    if delta + P < QW:
                        relu_evict(s[:, :, delta + P:QW],
                                   sp[:, :, delta + P:QW])
                else:
                    relu_evict(s[:, :, delta:QW], sp[:, :, delta:QW])
                state["s"][i] = s

            def av(i, qc, kb, kb_hi):
                if kb == 0:
                    state["y"][qc] = ypsum.tile([P, QW], F32, tag="y",
                                                name="yp")
                yp = state["y"][qc]
                delta = max(kb * P - qc * QW, 0)
                s = state["s"].pop(i)
                # the two heads accumulate into disjoint partition ranges of
                # one bank (different per-partition SRAMs, so concurrent
                # drains are safe); each runs its own start/stop group (the
                # sim's group checker can't see base partition -> skip)
                nc.tensor.matmul(
                    yp[0:64, delta:QW], vsb[:, kb, hp * P:hp * P + 64],
                    s[:, 0, delta:QW],
                    start=(kb == 0), stop=(kb == kb_hi),
                    tile_position=(0, 0), skip_group_check=True,
                )
                nc.tensor.matmul(
                    yp[64:128, delta:QW],
                    vsb[:, kb, hp * P + 64:hp * P + 128],
                    s[:, 1, delta:QW],
                    start=(kb == 0), stop=(kb == kb_hi),
                    tile_position=(0, 64), skip_group_check=True,
                )
                if kb == kb_hi:
                    yp = state["y"].pop(qc)
                    yt = ysb.tile([P, QW], BF16, tag="yt")
                    # one full-partition eviction: engine cost scales with
                    # free-dim cols only, so splitting by partition halves
                    # would double the engine time
                    nc.scalar.activation(yt[:], yp[:], AF.Copy)
                    nc.sync.dma_start(
                        yT[hp * P:(hp + 1) * P, qc * QW:(qc + 1) * QW],
                        yt[:])

            for i, (qc, kb, kb_hi) in enumerate(items):
                sc_fns.append(
                    lambda i=i, qc=qc, kb=kb, kb_hi=kb_hi: sc(i, qc, kb, kb_hi))
                av_fns.append(
                    lambda i=i, qc=qc, kb=kb, kb_hi=kb_hi: av(i, qc, kb, kb_hi))
            return sc_fns, av_fns

        # ---- schedule -----------------------------------------------------
        # Prelude: qk o-tiles for group 0 (heads 0-3), paced to DMA arrival.
        # x rows land ~[ct0, ct2, ct1, ct4, ct3, ct5] (scalar/gpsimd split),
        # w slots land s0..s3 in order, so slots 0-2 interleave mms in that
        # arrival order (3 PSUM bufs) and slot 3 runs once data is resident.
        for slot in range(4):
            qk_chain(slot)[1]()

        # Per-window background: v chains for the *current* group pop
        # aggressively (2/step) since av step kb needs v tile kb; the next
        # group's qk tiles follow a cycle budget.
        groups = [(0, 1), (2, 3), (4, 5)]
        LAG = 3
        for g, grp in enumerate(groups):
            streams = [attention_closures(hp) for hp in grp]
            front = deque(v_part(g, tt) for tt in range(TT))
            rest = deque()
            if g < 2:
                for slot in range(4 + 4 * g, 8 + 4 * g):
                    rest.append(qk_chain(slot))
            n = len(streams[0][0])
            nsteps = n + LAG
            budget = sum(c for c, _ in rest)
            spent = 0
            for i in range(nsteps):
                for _ in range(2):
                    if front:
                        front.popleft()[1]()
                while rest and spent < (i + 1) * budget // nsteps:
                    cyc, fn = rest.popleft()
                    fn()
                    spent += cyc
                if i < n:
                    for sc_fns, _ in streams:
                        sc_fns[i]()
                if i >= LAG:
                    for _, av_fns in streams:
                        av_fns[i - LAG]()
            while front:
                front.popleft()[1]()
            for cyc, fn in rest:
                fn()

    nc.compile()
    return nc


_CACHE = {}


def _get_module():
    if "nc" not in _CACHE:
        _CACHE["nc"] = build_module()
    return _CACHE["nc"]


def _prep_in_maps(x, W_attn, b_attn, T=1024, C=768, n_cores=8):
    bf = ml_dtypes.bfloat16
    CT = C // P
    OT = 2 * C // P
    WT = W_attn.astype(np.float32).T                                   # [C, 3C]
    # permute columns into PE consumption order: 12 qk o-tiles in OT_ORDER,
    # then the 6 v column tiles; lay out as [p, slot, ct, 128] so each slot
    # is 1.5KB-contiguous per partition for the DMA.
    cols = np.concatenate(
        [np.arange(ot * P, (ot + 1) * P) for ot in OT_ORDER]
        + [np.arange(2 * C, 3 * C)])
    wPm = WT[:, cols].reshape(CT, P, NSLOT, P).transpose(1, 2, 0, 3)
    wPm = np.ascontiguousarray(wPm).astype(bf)              # [P, 18, CT, 128]
    bqk = np.ascontiguousarray(
        b_attn[:2 * C].astype(np.float32).reshape(OT, P).T)            # [P, OT]
    bv = np.ascontiguousarray(
        np.tile(b_attn[2 * C:].astype(np.float32)[None, :], (P, 1)))   # [P, C]
    in_maps = []
    for c in range(n_cores):
        xT_b = np.ascontiguousarray(x[c].astype(np.float32).T).astype(bf)
        in_maps.append({"xT": xT_b, "wP": wPm, "bqk": bqk, "bv": bv})
    return in_maps


def run(x, W_attn, b_attn, trace=False):
    nc = _get_module()
    in_maps = _prep_in_maps(x, W_attn, b_attn)
    res = bass_utils.run_bass_kernel_spmd(
        nc, in_maps, core_ids=list(range(8)), trace=trace)
    y = np.stack([np.asarray(res.results[c]["yT"]).T for c in range(8)])
    return np.ascontiguousarray(y.astype(np.float32)), res


def kernel(x, W_attn, b_attn):
    y, _ = run(x, W_attn, b_attn, trace=False)
    return y


# revision 13
# speedup vs baseline: 1.0300x; 1.0130x over previous
"""Causal ReLU-attention block (qkv proj + per-head attention) on 8 trn2 cores.

Sharding: pure data-parallel over batch (B=8 -> 1 batch element per core).
Per-core: x_b [T,C] -> qkv -> scoresT = k q^T (row-tiled head pairs) ->
relu/causal-mask -> yT = v.T @ scoresT (col-tiled head pairs) -> DRAM yT [C,T].
Host side: transpose/cast shards in, transpose gather out.

Schedule: the qk projection for the first head-pair group runs as a short
prelude; every remaining projection matmul (later qk o-tiles + all v tiles)
is interleaved into the attention super-steps as background PE work so the
eviction engines (ACT/DVE) hide under matmul time instead of pacing the
kernel. The causal mask is fused into the diagonal-block eviction as
max(s,0) * mask{scale,0} (one DVE scalar_tensor_tensor), replacing the
per-block gpsimd affine_select. Input DMAs: x on the scalar queue (free
early), W on sync, host-permuted into PE consumption order so every DMA
has >=1.5KB contiguous runs; y DMAs ride sync after W is in.
"""

import sys
from collections import deque
from contextlib import ExitStack

sys.path.insert(0, "/opt/trn_rl_repo")

import ml_dtypes
import numpy as np

import concourse.bass as bass
import concourse.tile as tile
from concourse import bacc, bass_utils, mybir

P = 128
QW = 512  # t_q chunk width (PSUM bank = 512 fp32)

BF16 = mybir.dt.bfloat16
F32 = mybir.dt.float32
AF = mybir.ActivationFunctionType
ALU = mybir.AluOpType

# qk o-tiles (12 of them over [q|k] = 2C) in emission order: the q and k
# tiles of head pair hp are needed together, group (hp, hp+1) at a time.
OT_ORDER = [0, 6, 1, 7, 2, 8, 3, 9, 4, 10, 5, 11]
NSLOT = 18  # 12 qk slots + 6 v slots (128 cols each)
# W slot DMA order: group0 qk, v-g0, group1 qk, v-g1, group2 qk, v-g2
W_DMA_ORDER = [0, 1, 2, 3, 12, 13, 4, 5, 6, 7, 14, 15, 8, 9, 10, 11, 16, 17]


def build_module(T=1024, C=768, H=12, n_cores=8):
    """Build + compile the per-core Bass module (same program on all cores)."""
    hd = C // H
    assert hd == 64 and H % 2 == 0 and C % P == 0 and T % QW == 0
    CT = C // P            # contraction tiles over C
    TT = T // P            # t tiles
    NQC = T // QW          # q chunks
    NHP = H // 2           # head pairs
    scale = 1.0 / float(np.sqrt(hd))

    nc = bacc.Bacc("TRN2", target_bir_lowering=False, debug=False,
                   num_devices=n_cores)

    xT = nc.dram_tensor("xT", [C, T], BF16, kind="ExternalInput").ap()
    # W pre-permuted on host: [p, slot, ct, 128] (see _prep_in_maps)
    wP = nc.dram_tensor("wP", [P, NSLOT, CT, P], BF16, kind="ExternalInput").ap()
    bqk = nc.dram_tensor("bqk", [P, 2 * CT], F32, kind="ExternalInput").ap()
    bv = nc.dram_tensor("bv", [P, C], F32, kind="ExternalInput").ap()
    yT = nc.dram_tensor("yT", [C, T], BF16, kind="ExternalOutput").ap()

    xT3 = xT.rearrange("(ct p) t -> p ct t", p=P)

    with tile.TileContext(nc) as tc, ExitStack() as ctx:
        const = ctx.enter_context(tc.tile_pool(name="const", bufs=1))
        psum = ctx.enter_context(tc.tile_pool(name="psum", bufs=3, space="PSUM"))
        ypsum = ctx.enter_context(tc.tile_pool(name="ypsum", bufs=2, space="PSUM"))
        scb = ctx.enter_context(tc.tile_pool(name="scb", bufs=14))
        ysb = ctx.enter_context(tc.tile_pool(name="ysb", bufs=3))

        wt_sb = const.tile([P, NSLOT, CT, P], BF16)
        xt_sb = const.tile([P, CT, T], BF16)
        bqk_sb = const.tile([P, 2 * CT], F32)
        bv_sb = const.tile([P, C], F32)

        # ---- PE warm-up ---------------------------------------------------
        # A burst of junk matmuls so the PE runs continuously from ~7.3us
        # while the input DMAs land: the HAM activity window sees ~3.4us of
        # uninterrupted busy and un-throttles to 2.4GHz by ~10.7us, so all
        # real matmuls run warm. Without this, early DMA-paced bubbles keep
        # the PE at 1.2GHz for 18-25us (run-dependent).
        warm_sb = const.tile([P, QW], BF16)
        nc.gpsimd.memset(warm_sb[:], 0.0)
        warm_ps = ypsum.tile([P, QW], F32, tag="y", name="warm")
        for _ in range(9):
            nc.tensor.matmul(warm_ps[:], warm_sb[:, 0:P], warm_sb[:],
                             start=True, stop=True)

        # ---- input DMAs --------------------------------------------------
        # Per-queue DMA bandwidth is only ~100-160GB/s, so the early feed is
        # split across all three DMA-capable queues: x full rows (2KB
        # contiguous runs) alternate scalar (even ct) / gpsimd (odd ct) in
        # consumption order; W slots ride sync, then sync later takes y.
        for qc in range(NQC):
            for ct in range(CT):
                eng = nc.scalar if ct % 2 == 0 else nc.gpsimd
                eng.dma_start(xt_sb[:, ct, qc * QW:(qc + 1) * QW],
                              xT3[:, ct, qc * QW:(qc + 1) * QW])
        nc.scalar.dma_start(bqk_sb[:], bqk[:])
        nc.scalar.dma_start(bv_sb[:], bv[:])
        for s in W_DMA_ORDER:
            nc.sync.dma_start(wt_sb[:, s], wP[:, s])

        # causal mask for diagonal 128x128 strips of scoresT [t_k, t_q]:
        # mask[p, j] = scale where j >= p else 0 (relu(scale*s)*m ==
        # max(s,0)*(scale*m) since scale > 0)
        mask_sc = const.tile([P, P], BF16)
        nc.gpsimd.memset(mask_sc[:], scale)
        nc.gpsimd.affine_select(
            mask_sc[:], mask_sc[:], pattern=[[1, P]],
            compare_op=ALU.is_ge, fill=0.0, base=0, channel_multiplier=-1)

        qkT = const.tile([P, 2 * CT, T], BF16)   # o-tiles: q = 0..CT-1, k = CT..
        vsb = const.tile([P, TT, C], BF16)       # v in natural [t, o] layout

        evict = [0]

        def relu_evict(dst, src):
            # relu(scale * s): PSUM -> SBUF bf16; ACT is 1.25x faster than
            # DVE so give it 3 of every 5 (DVE also owns the fused diagonal
            # evictions and v bias adds)
            if evict[0] % 5 < 3:
                nc.scalar.activation(dst, src, AF.Relu, scale=scale)
            else:
                nc.vector.tensor_scalar(dst, src, scale, 0.0, ALU.mult, ALU.max)
            evict[0] += 1

        # ---- background (projection) chains ------------------------------
        # Each chain is atomic (alloc ... evict in one pop): a PSUM buf may
        # only be held across instructions emitted before its eviction, else
        # the round-robin pool can deadlock the tensor FIFO.
        def qk_chain(slot):
            """One qk o-tile: 2 qc x CT-deep accumulation chains into one
            PSUM tile, evicted by ACT with the fused per-partition bias."""
            ot = OT_ORDER[slot]

            def fn():
                ps = psum.tile([P, NQC, QW], F32, tag="blk", name="qk_ps")
                for qc in range(NQC):
                    for ct in range(CT):
                        nc.tensor.matmul(
                            ps[:, qc],
                            wt_sb[:, slot, ct, :],
                            xt_sb[:, ct, qc * QW:(qc + 1) * QW],
                            start=(ct == 0), stop=(ct == CT - 1),
                        )
                nc.scalar.activation(
                    qkT[:, ot], ps.rearrange("p a b -> p (a b)"),
                    AF.Identity, bias=bqk_sb[:, ot:ot + 1])

            return (NQC * CT * QW, fn)

        def v_part(g, tt):
            """v columns [g*256, (g+1)*256) for one t-tile: 6-deep chain +
            bias add (DVE; PSUM-reading tensor_tensor is DVE-only)."""
            def fn():
                ps = psum.tile([P, 2, P], F32, tag="blk", name="v_ps")
                for ct in range(CT):
                    nc.tensor.matmul(
                        ps[:],
                        xt_sb[:, ct, tt * P:(tt + 1) * P],
                        wt_sb[:, 12 + 2 * g:14 + 2 * g, ct, :],
                        start=(ct == 0), stop=(ct == CT - 1),
                    )
                nc.vector.tensor_tensor(
                    vsb[:, tt, g * 2 * P:(g + 1) * 2 * P],
                    ps.rearrange("p a b -> p (a b)"),
                    bv_sb[:, g * 2 * P:(g + 1) * 2 * P], ALU.add)
            return (CT * 2 * P, fn)

        # ---- attention ----------------------------------------------------
        def attention_closures(hp):
            """Parallel (scores, att@v) emission closures per block step for
            one head pair; the interleaver runs att@v a full super-step
            behind its scores so the FIFO PE queue always has ready work."""
            items = []
            for qc in range(NQC):
                kb_hi = min((qc * QW + QW - 1) // P, TT - 1)
                for kb in range(kb_hi + 1):
                    items.append((qc, kb, kb_hi))
            state = {"s": {}, "y": {}}
            sc_fns, av_fns = [], []

            def sc(i, qc, kb, kb_hi):
                delta = max(kb * P - qc * QW, 0)   # first valid t_q col
                sp = psum.tile([P, 2, QW], F32, tag="blk", name="s_ps")
                for h, ppos in ((0, (0, 0)), (1, (64, 0))):
                    nc.tensor.matmul(
                        sp[:, h, delta:QW],
                        qkT[h * 64:(h + 1) * 64, CT + hp,
                            kb * P:(kb + 1) * P],
                        qkT[h * 64:(h + 1) * 64, hp,
                            qc * QW + delta:(qc + 1) * QW],
                        start=True, stop=True, tile_position=ppos,
                    )
                s = scb.tile([P, 2, QW], BF16, tag="s")
                if kb * P >= qc * QW:
                    # diagonal block: fused relu+scale+causal-mask on the
                    # first P cols (row p only masks j' < p < P)
                    nc.vector.scalar_tensor_tensor(
                        s[:, :, delta:delta + P],
                        sp[:, :, delta:delta + P],
                        0.0,
                        mask_sc[:, None, :].to_broadcast((P, 2, P)),
                        ALU.max, ALU.mult)
                    if delta + P < QW:
                        relu_evict(s[:, :, delta + P:QW],
                                   sp[:, :, delta + P:QW])
                else:
                    relu_evict(s[:, :, delta:QW], sp[:, :, delta:QW])
                state["s"][i] = s

            def av(i, qc, kb, kb_hi):
                if kb == 0:
                    state["y"][qc] = ypsum.tile([P, QW], F32, tag="y",
                                                name="yp")
                yp = state["y"][qc]
                delta = max(kb * P - qc * QW, 0)
                s = state["s"].pop(i)
                # the two heads accumulate into disjoint partition ranges of
                # one bank (different per-partition SRAMs, so concurrent
                # drains are safe); each runs its own start/stop group (the
                # sim's group checker can't see base partition -> skip)
                nc.tensor.matmul(
                    yp[0:64, delta:QW], vsb[:, kb, hp * P:hp * P + 64],
                    s[:, 0, delta:QW],
                    start=(kb == 0), stop=(kb == kb_hi),
                    tile_position=(0, 0), skip_group_check=True,
                )
                nc.tensor.matmul(
                    yp[64:128, delta:QW],
                    vsb[:, kb, hp * P + 64:hp * P + 128],
                    s[:, 1, delta:QW],
                    start=(kb == 0), stop=(kb == kb_hi),
                    tile_position=(0, 64), skip_group_check=True,
                )
                if kb == kb_hi:
                    yp = state["y"].pop(qc)
                    yt = ysb.tile([P, QW], BF16, tag="yt")
                    # one full-partition eviction: engine cost scales with
                    # free-dim cols only, so splitting by partition halves
                    # would double the engine time
                    nc.scalar.activation(yt[:], yp[:], AF.Copy)
                    nc.sync.dma_start(
                        yT[hp * P:(hp + 1) * P, qc * QW:(qc + 1) * QW],
                        yt[:])

            for i, (qc, kb, kb_hi) in enumerate(items):
                sc_fns.append(
                    lambda i=i, qc=qc, kb=kb, kb_hi=kb_hi: sc(i, qc, kb, kb_hi))
                av_fns.append(
                    lambda i=i, qc=qc, kb=kb, kb_hi=kb_hi: av(i, qc, kb, kb_hi))
            return sc_fns, av_fns

        # ---- schedule -----------------------------------------------------
        # Prelude: qk o-tiles for group 0 (heads 0-3), paced to DMA arrival.
        # x rows land ~[ct0, ct2, ct1, ct4, ct3, ct5] (scalar/gpsimd split),
        # w slots land s0..s3 in order, so slots 0-2 interleave mms in that
        # arrival order (3 PSUM bufs) and slot 3 runs once data is resident.
        for slot in range(4):
            qk_chain(slot)[1]()

        # Per-window background: v chains for the *current* group pop
        # aggressively (2/step) since av step kb needs v tile kb; the next
        # group's qk tiles follow a cycle budget.
        groups = [(0, 1), (2, 3), (4, 5)]
        LAG = 3
        for g, grp in enumerate(groups):
            streams = [attention_closures(hp) for hp in grp]
            front = deque(v_part(g, tt) for tt in range(TT))
            rest = deque()
            if g < 2:
                for slot in range(4 + 4 * g, 8 + 4 * g):
                    rest.append(qk_chain(slot))
            n = len(streams[0][0])
            nsteps = n + LAG
            budget = sum(c for c, _ in rest)
            spent = 0
            for i in range(nsteps):
                for _ in range(2):
                    if front:
                        front.popleft()[1]()
                while rest and spent < (i + 1) * budget // nsteps:
                    cyc, fn = rest.popleft()
                    fn()
                    spent += cyc
                if i < n:
                    for sc_fns, _ in streams:
                        sc_fns[i]()
                if i >= LAG:
                    for _, av_fns in streams:
                        av_fns[i - LAG]()
            while front:
                front.popleft()[1]()
            for cyc, fn in rest:
                fn()

    nc.compile()
    return nc


_CACHE = {}


def _get_module():
    if "nc" not in _CACHE:
        _CACHE["nc"] = build_module()
    return _CACHE["nc"]


def _prep_in_maps(x, W_attn, b_attn, T=1024, C=768, n_cores=8):
    bf = ml_dtypes.bfloat16
    CT = C // P
    OT = 2 * C // P
    WT = W_attn.astype(np.float32).T                                   # [C, 3C]
    # permute columns into PE consumption order: 12 qk o-tiles in OT_ORDER,
    # then the 6 v column tiles; lay out as [p, slot, ct, 128] so each slot
    # is 1.5KB-contiguous per partition for the DMA.
    cols = np.concatenate(
        [np.arange(ot * P, (ot + 1) * P) for ot in OT_ORDER]
        + [np.arange(2 * C, 3 * C)])
    wPm = WT[:, cols].reshape(CT, P, NSLOT, P).transpose(1, 2, 0, 3)
    wPm = np.ascontiguousarray(wPm).astype(bf)              # [P, 18, CT, 128]
    bqk = np.ascontiguousarray(
        b_attn[:2 * C].astype(np.float32).reshape(OT, P).T)            # [P, OT]
    bv = np.ascontiguousarray(
        np.tile(b_attn[2 * C:].astype(np.float32)[None, :], (P, 1)))   # [P, C]
    in_maps = []
    for c in range(n_cores):
        xT_b = np.ascontiguousarray(x[c].astype(np.float32).T).astype(bf)
        in_maps.append({"xT": xT_b, "wP": wPm, "bqk": bqk, "bv": bv})
    return in_maps


def run(x, W_attn, b_attn, trace=False):
    nc = _get_module()
    in_maps = _prep_in_maps(x, W_attn, b_attn)
    res = bass_utils.run_bass_kernel_spmd(
        nc, in_maps, core_ids=list(range(8)), trace=trace)
    y = np.stack([np.asarray(res.results[c]["yT"]).T for c in range(8)])
    return np.ascontiguousarray(y.astype(np.float32)), res


def kernel(x, W_attn, b_attn):
    y, _ = run(x, W_attn, b_attn, trace=False)
    return y


# revision 14
# speedup vs baseline: 1.0529x; 1.0223x over previous
"""Causal ReLU-attention block (qkv proj + per-head attention) on 8 trn2 cores.

Sharding: pure data-parallel over batch (B=8 -> 1 batch element per core).
Per-core: x_b [T,C] -> qkv -> scoresT = k q^T (row-tiled head pairs) ->
relu/causal-mask -> yT = v.T @ scoresT (col-tiled head pairs) -> DRAM yT [C,T].
Host side: transpose/cast shards in, transpose gather out.

Schedule: the qk projection for the first head-pair group runs as a short
prelude; every remaining projection matmul (later qk o-tiles + all v tiles)
is interleaved into the attention super-steps as background PE work so the
eviction engines (ACT/DVE) hide under matmul time instead of pacing the
kernel. The causal mask is fused into the diagonal-block eviction as
max(s,0) * mask{scale,0} (one DVE scalar_tensor_tensor), replacing the
per-block gpsimd affine_select. Input DMAs: x on the scalar queue (free
early), W on sync, host-permuted into PE consumption order so every DMA
has >=1.5KB contiguous runs; y DMAs ride sync after W is in.
"""

import sys
from collections import deque
from contextlib import ExitStack

sys.path.insert(0, "/opt/trn_rl_repo")

import ml_dtypes
import numpy as np

import concourse.bass as bass
import concourse.tile as tile
from concourse import bacc, bass_utils, mybir

P = 128
QW = 512  # t_q chunk width (PSUM bank = 512 fp32)

BF16 = mybir.dt.bfloat16
F32 = mybir.dt.float32
AF = mybir.ActivationFunctionType
ALU = mybir.AluOpType

# qk o-tiles (12 of them over [q|k] = 2C) in emission order: the q and k
# tiles of head pair hp are needed together, group (hp, hp+1) at a time.
OT_ORDER = [0, 6, 1, 7, 2, 8, 3, 9, 4, 10, 5, 11]
NSLOT = 18  # 12 qk slots + 6 v slots (128 cols each)
# W slot DMA order: group0 qk, v-g0, group1 qk, v-g1, group2 qk, v-g2
W_DMA_ORDER = [0, 1, 2, 3, 12, 13, 4, 5, 6, 7, 14, 15, 8, 9, 10, 11, 16, 17]


def build_module(T=1024, C=768, H=12, n_cores=8):
    """Build + compile the per-core Bass module (same program on all cores)."""
    hd = C // H
    assert hd == 64 and H % 2 == 0 and C % P == 0 and T % QW == 0
    CT = C // P            # contraction tiles over C
    TT = T // P            # t tiles
    NQC = T // QW          # q chunks
    NHP = H // 2           # head pairs
    scale = 1.0 / float(np.sqrt(hd))

    nc = bacc.Bacc("TRN2", target_bir_lowering=False, debug=False,
                   num_devices=n_cores)

    xT = nc.dram_tensor("xT", [C, T], BF16, kind="ExternalInput").ap()
    # W pre-permuted on host: [p, slot, ct, 128] (see _prep_in_maps)
    wP = nc.dram_tensor("wP", [P, NSLOT, CT, P], BF16, kind="ExternalInput").ap()
    bqk = nc.dram_tensor("bqk", [P, 2 * CT], F32, kind="ExternalInput").ap()
    bv = nc.dram_tensor("bv", [P, C], F32, kind="ExternalInput").ap()
    yT = nc.dram_tensor("yT", [C, T], BF16, kind="ExternalOutput").ap()

    xT3 = xT.rearrange("(ct p) t -> p ct t", p=P)

    with tile.TileContext(nc) as tc, ExitStack() as ctx:
        const = ctx.enter_context(tc.tile_pool(name="const", bufs=1))
        psum = ctx.enter_context(tc.tile_pool(name="psum", bufs=3, space="PSUM"))
        ypsum = ctx.enter_context(tc.tile_pool(name="ypsum", bufs=2, space="PSUM"))
        scb = ctx.enter_context(tc.tile_pool(name="scb", bufs=14))
        ysb = ctx.enter_context(tc.tile_pool(name="ysb", bufs=3))

        wt_sb = const.tile([P, NSLOT, CT, P], BF16)
        xt_sb = const.tile([P, CT, T], BF16)
        bqk_sb = const.tile([P, 2 * CT], F32)
        bv_sb = const.tile([P, C], F32)

        # ---- PE warm-up ---------------------------------------------------
        # A burst of junk matmuls so the PE runs continuously from ~7.3us
        # while the input DMAs land: the HAM activity window sees ~3.4us of
        # uninterrupted busy and un-throttles to 2.4GHz by ~10.7us, so all
        # real matmuls run warm. Without this, early DMA-paced bubbles keep
        # the PE at 1.2GHz for 18-25us (run-dependent).
        warm_sb = const.tile([P, QW], BF16)
        nc.gpsimd.memset(warm_sb[:], 0.0)
        warm_ps = ypsum.tile([P, QW], F32, tag="y", name="warm")
        for _ in range(9):
            nc.tensor.matmul(warm_ps[:], warm_sb[:, 0:P], warm_sb[:],
                             start=True, stop=True)

        # ---- input DMAs --------------------------------------------------
        # Per-queue DMA bandwidth is only ~100-160GB/s and x (1.5MB) gates
        # the whole prelude, so x chunks are spread over ALL THREE DMA
        # queues in consumption order (ct%3 -> scalar/gpsimd/sync); sync
        # delivers w slot 0 first (first chain's stationary), the remaining
        # W slots after x, then takes the y outputs.
        nc.scalar.dma_start(bqk_sb[:], bqk[:])
        nc.sync.dma_start(wt_sb[:, 0], wP[:, 0])
        for qc in range(NQC):
            for ct in range(CT):
                eng = (nc.scalar, nc.gpsimd, nc.sync)[ct % 3]
                eng.dma_start(xt_sb[:, ct, qc * QW:(qc + 1) * QW],
                              xT3[:, ct, qc * QW:(qc + 1) * QW])
        nc.scalar.dma_start(bv_sb[:], bv[:])
        for s in W_DMA_ORDER[1:]:
            nc.sync.dma_start(wt_sb[:, s], wP[:, s])

        # causal mask for diagonal 128x128 strips of scoresT [t_k, t_q]:
        # mask[p, j] = scale where j >= p else 0 (relu(scale*s)*m ==
        # max(s,0)*(scale*m) since scale > 0)
        mask_sc = const.tile([P, P], BF16)
        nc.gpsimd.memset(mask_sc[:], scale)
        nc.gpsimd.affine_select(
            mask_sc[:], mask_sc[:], pattern=[[1, P]],
            compare_op=ALU.is_ge, fill=0.0, base=0, channel_multiplier=-1)

        qkT = const.tile([P, 2 * CT, T], BF16)   # o-tiles: q = 0..CT-1, k = CT..
        vsb = const.tile([P, TT, C], BF16)       # v in natural [t, o] layout

        evict = [0]

        def relu_evict(dst, src):
            # relu(scale * s): PSUM -> SBUF bf16; ACT is 1.25x faster than
            # DVE so give it 3 of every 5 (DVE also owns the fused diagonal
            # evictions and v bias adds)
            if evict[0] % 5 < 3:
                nc.scalar.activation(dst, src, AF.Relu, scale=scale)
            else:
                nc.vector.tensor_scalar(dst, src, scale, 0.0, ALU.mult, ALU.max)
            evict[0] += 1

        # ---- background (projection) chains ------------------------------
        # Each chain is atomic (alloc ... evict in one pop): a PSUM buf may
        # only be held across instructions emitted before its eviction, else
        # the round-robin pool can deadlock the tensor FIFO.
        def qk_chain(slot):
            """One qk o-tile: 2 qc x CT-deep accumulation chains into one
            PSUM tile, evicted by ACT with the fused per-partition bias."""
            ot = OT_ORDER[slot]

            def fn():
                ps = psum.tile([P, NQC, QW], F32, tag="blk", name="qk_ps")
                for qc in range(NQC):
                    for ct in range(CT):
                        nc.tensor.matmul(
                            ps[:, qc],
                            wt_sb[:, slot, ct, :],
                            xt_sb[:, ct, qc * QW:(qc + 1) * QW],
                            start=(ct == 0), stop=(ct == CT - 1),
                        )
                nc.scalar.activation(
                    qkT[:, ot], ps.rearrange("p a b -> p (a b)"),
                    AF.Identity, bias=bqk_sb[:, ot:ot + 1])

            return (NQC * CT * QW, fn)

        def v_part(g, tt):
            """v columns [g*256, (g+1)*256) for one t-tile: 6-deep chain +
            bias add (DVE; PSUM-reading tensor_tensor is DVE-only)."""
            def fn():
                ps = psum.tile([P, 2, P], F32, tag="blk", name="v_ps")
                for ct in range(CT):
                    nc.tensor.matmul(
                        ps[:],
                        xt_sb[:, ct, tt * P:(tt + 1) * P],
                        wt_sb[:, 12 + 2 * g:14 + 2 * g, ct, :],
                        start=(ct == 0), stop=(ct == CT - 1),
                    )
                nc.vector.tensor_tensor(
                    vsb[:, tt, g * 2 * P:(g + 1) * 2 * P],
                    ps.rearrange("p a b -> p (a b)"),
                    bv_sb[:, g * 2 * P:(g + 1) * 2 * P], ALU.add)
            return (CT * 2 * P, fn)

        # ---- attention ----------------------------------------------------
        def attention_closures(hp):
            """Parallel (scores, att@v) emission closures per block step for
            one head pair; the interleaver runs att@v a full super-step
            behind its scores so the FIFO PE queue always has ready work."""
            items = []
            for qc in range(NQC):
                kb_hi = min((qc * QW + QW - 1) // P, TT - 1)
                for kb in range(kb_hi + 1):
                    items.append((qc, kb, kb_hi))
            state = {"s": {}, "y": {}}
            sc_fns, av_fns = [], []

            def sc(i, qc, kb, kb_hi):
                delta = max(kb * P - qc * QW, 0)   # first valid t_q col
                sp = psum.tile([P, 2, QW], F32, tag="blk", name="s_ps")
                for h, ppos in ((0, (0, 0)), (1, (64, 0))):
                    nc.tensor.matmul(
                        sp[:, h, delta:QW],
                        qkT[h * 64:(h + 1) * 64, CT + hp,
                            kb * P:(kb + 1) * P],
                        qkT[h * 64:(h + 1) * 64, hp,
                            qc * QW + delta:(qc + 1) * QW],
                        start=True, stop=True, tile_position=ppos,
                    )
                s = scb.tile([P, 2, QW], BF16, tag="s")
                if kb * P >= qc * QW:
                    # diagonal block: fused relu+scale+causal-mask on the
                    # first P cols (row p only masks j' < p < P)
                    nc.vector.scalar_tensor_tensor(
                        s[:, :, delta:delta + P],
                        sp[:, :, delta:delta + P],
                        0.0,
                        mask_sc[:, None, :].to_broadcast((P, 2, P)),
                        ALU.max, ALU.mult)
                    if delta + P < QW:
                        relu_evict(s[:, :, delta + P:QW],
                                   sp[:, :, delta + P:QW])
                else:
                    relu_evict(s[:, :, delta:QW], sp[:, :, delta:QW])
                state["s"][i] = s

            def av(i, qc, kb, kb_hi):
                if kb == 0:
                    state["y"][qc] = ypsum.tile([P, QW], F32, tag="y",
                                                name="yp")
                yp = state["y"][qc]
                delta = max(kb * P - qc * QW, 0)
                s = state["s"].pop(i)
                # the two heads accumulate into disjoint partition ranges of
                # one bank (different per-partition SRAMs, so concurrent
                # drains are safe); each runs its own start/stop group (the
                # sim's group checker can't see base partition -> skip)
                nc.tensor.matmul(
                    yp[0:64, delta:QW], vsb[:, kb, hp * P:hp * P + 64],
                    s[:, 0, delta:QW],
                    start=(kb == 0), stop=(kb == kb_hi),
                    tile_position=(0, 0), skip_group_check=True,
                )
                nc.tensor.matmul(
                    yp[64:128, delta:QW],
                    vsb[:, kb, hp * P + 64:hp * P + 128],
                    s[:, 1, delta:QW],
                    start=(kb == 0), stop=(kb == kb_hi),
                    tile_position=(0, 64), skip_group_check=True,
                )
                if kb == kb_hi:
                    yp = state["y"].pop(qc)
                    yt = ysb.tile([P, QW], BF16, tag="yt")
                    # one full-partition eviction: engine cost scales with
                    # free-dim cols only, so splitting by partition halves
                    # would double the engine time
                    nc.scalar.activation(yt[:], yp[:], AF.Copy)
                    nc.sync.dma_start(
                        yT[hp * P:(hp + 1) * P, qc * QW:(qc + 1) * QW],
                        yt[:])

            for i, (qc, kb, kb_hi) in enumerate(items):
                sc_fns.append(
                    lambda i=i, qc=qc, kb=kb, kb_hi=kb_hi: sc(i, qc, kb, kb_hi))
                av_fns.append(
                    lambda i=i, qc=qc, kb=kb, kb_hi=kb_hi: av(i, qc, kb, kb_hi))
            return sc_fns, av_fns

        # ---- schedule -----------------------------------------------------
        # Prelude: qk o-tiles for group 0 (heads 0-3), paced to DMA arrival.
        # x rows land ~[ct0, ct2, ct1, ct4, ct3, ct5] (scalar/gpsimd split),
        # w slots land s0..s3 in order, so slots 0-2 interleave mms in that
        # arrival order (3 PSUM bufs) and slot 3 runs once data is resident.
        for slot in range(4):
            qk_chain(slot)[1]()

        # Per-window background: v chains for the *current* group pop
        # aggressively (2/step) since av step kb needs v tile kb; the next
        # group's qk tiles follow a cycle budget.
        groups = [(0, 1), (2, 3), (4, 5)]
        LAG = 3
        for g, grp in enumerate(groups):
            streams = [attention_closures(hp) for hp in grp]
            front = deque(v_part(g, tt) for tt in range(TT))
            rest = deque()
            if g < 2:
                for slot in range(4 + 4 * g, 8 + 4 * g):
                    rest.append(qk_chain(slot))
            n = len(streams[0][0])
            nsteps = n + LAG
            budget = sum(c for c, _ in rest)
            spent = 0
            for i in range(nsteps):
                for _ in range(2):
                    if front:
                        front.popleft()[1]()
                while rest and spent < (i + 1) * budget // nsteps:
                    cyc, fn = rest.popleft()
                    fn()
                    spent += cyc
                if i < n:
                    for sc_fns, _ in streams:
                        sc_fns[i]()
                if i >= LAG:
                    for _, av_fns in streams:
                        av_fns[i - LAG]()
            while front:
                front.popleft()[1]()
            for cyc, fn in rest:
                fn()

    nc.compile()
    return nc


_CACHE = {}


def _get_module():
    if "nc" not in _CACHE:
        _CACHE["nc"] = build_module()
    return _CACHE["nc"]


def _prep_in_maps(x, W_attn, b_attn, T=1024, C=768, n_cores=8):
    bf = ml_dtypes.bfloat16
    CT = C // P
    OT = 2 * C // P
    WT = W_attn.astype(np.float32).T                                   # [C, 3C]
    # permute columns into PE consumption order: 12 qk o-tiles in OT_ORDER,
    # then the 6 v column tiles; lay out as [p, slot, ct, 128] so each slot
    # is 1.5KB-contiguous per partition for the DMA.
    cols = np.concatenate(
        [np.arange(ot * P, (ot + 1) * P) for ot in OT_ORDER]
        + [np.arange(2 * C, 3 * C)])
    wPm = WT[:, cols].reshape(CT, P, NSLOT, P).transpose(1, 2, 0, 3)
    wPm = np.ascontiguousarray(wPm).astype(bf)              # [P, 18, CT, 128]
    bqk = np.ascontiguousarray(
        b_attn[:2 * C].astype(np.float32).reshape(OT, P).T)            # [P, OT]
    bv = np.ascontiguousarray(
        np.tile(b_attn[2 * C:].astype(np.float32)[None, :], (P, 1)))   # [P, C]
    in_maps = []
    for c in range(n_cores):
        xT_b = np.ascontiguousarray(x[c].astype(np.float32).T).astype(bf)
        in_maps.append({"xT": xT_b, "wP": wPm, "bqk": bqk, "bv": bv})
    return in_maps


def run(x, W_attn, b_attn, trace=False):
    nc = _get_module()
    in_maps = _prep_in_maps(x, W_attn, b_attn)
    res = bass_utils.run_bass_kernel_spmd(
        nc, in_maps, core_ids=list(range(8)), trace=trace)
    y = np.stack([np.asarray(res.results[c]["yT"]).T for c in range(8)])
    return np.ascontiguousarray(y.astype(np.float32)), res


def kernel(x, W_attn, b_attn):
    y, _ = run(x, W_attn, b_attn, trace=False)
    return y
